# revision 55
# baseline (speedup 1.0000x reference)
"""DCSLoss Trainium2 kernel (8-core SPMD Bass/Tile).

Math (reference collapses to class level since samples come in contiguous
blocks of K=8 per class):
  xn        = row-normalized inputs                             [N, D]
  cc[c]     = mean of xn rows of class c                        [C, D]
  dist_pc_i = ||xn_i - cc_ci/||cc_ci|| ||  -> mean over N
  Dc[a,b]   = sqrt(max(|cc_a|^2+|cc_b|^2-2 cc_a.cc_b, 1e-12))   [C, C]
  dist_an_a = K * sum_{b!=a} relu(M2 - Dc[a,b]) / (N - K) -> mean over C
  loss      = dist_pc_mean + dist_an_mean

Sharding: core r owns samples [r*1024, (r+1)*1024) = classes [r*128, (r+1)*128).
Each core computes its local class centers, all-gathers a [128, 257] bf16
table (transposed centers | squared norms), computes its 128x1024 slice of the
hinge matrix plus its local dist_pc terms, and reduces to 3 partial scalars.
Host sums the 8x3 partials. The Dc diagonal is included on-device (evaluates
to M2 - sqrt(fp-noise) ~= M2) and subtracted as C*M2 on host.

AG table layout: per-rank row p, cols h*128+c hold ccT[h*128+p, c] (c = local
class), col 256 holds |cc_p|^2 — so the gathered [1024, 257] table yields the
[256, 1024] transposed center matrix via two strided DMAs and the squared-norm
row via a gather DMA, with no post-AG transposes.
"""

import os
import sys

import numpy as np

sys.path.insert(0, "/opt/trn_rl_repo")

import ml_dtypes  # noqa: E402

import concourse.bacc as bacc  # noqa: E402
import concourse.bass as bass  # noqa: E402
import concourse.mybir as mybir  # noqa: E402
import concourse.tile as tile  # noqa: E402
from concourse.bass_utils import run_bass_kernel_spmd  # noqa: E402

N, D, K = 8192, 256, 8
NCORES = 8
NL = N // NCORES  # 1024 samples per core
TP = 128  # SBUF partitions
NT = NL // TP  # 8 sample tiles per core
CPT = TP // K  # 16 classes per sample tile
C = N // K  # 1024 classes
MARGIN2 = 0.7
AG_COLS = 2 * TP + 1  # 257: two transposed 128-col blocks + squared norms
F32 = mybir.dt.float32
BF16 = mybir.dt.bfloat16
FP8 = mybir.dt.float8e4

_CACHE: dict = {}


def _build_nc() -> bass.Bass:
    AluOp = mybir.AluOpType
    Act = mybir.ActivationFunctionType
    stage = os.environ.get("DCS_STAGE", "all")  # ablation knob for sim

    nc = bacc.Bacc(target_bir_lowering=True)
    x = nc.dram_tensor("x", [NL, D], F32, kind="ExternalInput")
    wbig = nc.dram_tensor("wbig", [TP, 2 * NT, TP], BF16, kind="ExternalInput")
    eyeb = nc.dram_tensor("eyeb", [TP, TP], BF16, kind="ExternalInput")
    out = nc.dram_tensor("partials", [TP, 3], F32, kind="ExternalOutput")

    with tile.TileContext(nc) as tc:
        with (
            tc.tile_pool(name="const", bufs=1) as cp,
            tc.tile_pool(name="work", bufs=2) as wp,
            tc.tile_pool(name="psum", bufs=1, space="PSUM") as pp,
            tc.tile_pool(name="dram", bufs=1, space="DRAM") as dp,
        ):
            # --- constants ---
            WBIG = cp.tile([TP, 2 * NT, TP], BF16)
            EYE = cp.tile([TP, TP], BF16)
            nc.gpsimd.dma_start(EYE[:], eyeb[:])
            ones = cp.tile([TP, 1], F32)
            nc.vector.memset(ones[:], 1.0)
            ones_b = cp.tile([1, TP], FP8)
            nc.vector.memset(ones_b[:], 1.0)
            b_m2 = cp.tile([TP, 1], F32)
            nc.vector.memset(b_m2[:], MARGIN2)

            # Pin the ACT function table to sqrt_and_others (holds sqrt,
            # square, relu, copy) so no mid-kernel table reload happens.
            actp = cp.tile([TP, 1], F32)
            nc.scalar.sqrt(actp[:], ones[:])

            # --- load local samples in four chunks so the per-tile pipeline
            # starts as soon as the first chunk lands: Xs[q][p, u, :] =
            # x[(2q+u)*128 + p, :] ---
            xr = x[:].rearrange("(t p) d -> p t d", p=TP)
            Xs = []
            for q in range(4):
                Xq = wp.tile([TP, 2, D], F32, tag=f"X{q}")
                eng = nc.sync if q % 2 == 0 else nc.scalar
                eng.dma_start(Xq[:], xr[:, 2 * q : 2 * q + 2, :])
                Xs.append(Xq)

            def Xt(t):
                return Xs[t // 2][:, t % 2, :]

            # issued after the X chunks: PE needs WBIG only once tile 0 is
            # normalized, so X gets the head of the DMA queue
            nc.sync.dma_start(WBIG[:], wbig[:])

            # --- per-tile pipeline: squared norms (ACT) -> sqrt (ACT) ->
            # reciprocal (DVE) -> normalize (DVE, bf16) -> center matmul (PE).
            # Tiles flow through the five engines instead of barriering on the
            # whole batch. wbig[:, t, :] is the static (onehot/8) block for
            # sample tile t, pre-shifted so all 8 matmuls accumulate into one
            # [128, D] PSUM tile at partition 0. ---
            n2 = wp.tile([TP, NT], F32, tag="n2")
            nrm = wp.tile([TP, NT], F32, tag="nrm")
            rinv = wp.tile([TP, NT], F32, tag="rinv")
            XN = wp.tile([TP, NT, D], BF16, tag="XN")
            ccp = pp.tile([TP, D], F32, tag="ccp")
            for t in range(NT):
                if t < 5:
                    scr = wp.tile([TP, D], BF16, tag="scr")
                    nc.scalar.activation(
                        scr[:], Xt(t), Act.Square,
                        accum_out=n2[:, t : t + 1],
                    )
                else:
                    # GpSimd is otherwise idle: offload the square there and
                    # reduce on DVE, freeing ACT for the earlier tiles.
                    scrp = wp.tile([TP, D], BF16, tag=f"scrp{t}")
                    nc.gpsimd.tensor_tensor(scrp[:], Xt(t), Xt(t), AluOp.mult)
                    nc.vector.tensor_reduce(
                        n2[:, t : t + 1], scrp[:],
                        axis=mybir.AxisListType.X, op=AluOp.add,
                    )
                nc.scalar.activation(
                    nrm[:, t : t + 1], n2[:, t : t + 1], Act.Sqrt
                )
                nc.vector.reciprocal(rinv[:, t : t + 1], nrm[:, t : t + 1])
                nc.vector.tensor_scalar_mul(
                    XN[:, t, :], Xt(t), rinv[:, t : t + 1]
                )
                nc.tensor.matmul(
                    ccp[:], WBIG[:, t, :], XN[:, t, :],
                    start=(t == 0), stop=(t == NT - 1),
                )
            # --- bf16 centers straight from PSUM; squared norms and the
            # normalized centers both derive from the bf16 values, keeping
            # the hinge diagonal consistent. ---
            ccb = wp.tile([TP, D], BF16, tag="ccb")
            nc.vector.tensor_copy(ccb[:], ccp[:])
            # sqc = |fp8(cc)|^2, from the SAME fp8 values the gathered table
            # ships: the hinge then computes exact distances between the
            # fp8-rounded centers (|a|^2+|b|^2-2ab = |a-b|^2), so the fp8
            # rounding error enters the loss only at second order.
            cc8 = wp.tile([TP, D], FP8, tag="cc8")
            nc.scalar.copy(cc8[:], ccb[:])
            sqc = wp.tile([TP, 1], F32, tag="sqc")
            scr2 = wp.tile([TP, D], BF16, tag="scr")
            nc.scalar.activation(scr2[:], cc8[:], Act.Square, accum_out=sqc[:])
            sqn = wp.tile([TP, 1], F32, tag="sqn")
            nc.scalar.activation(sqn[:], sqc[:], Act.Sqrt)
            rc = wp.tile([TP, 1], F32, tag="rc")
            nc.vector.reciprocal(rc[:], sqn[:])
            cn = wp.tile([TP, D], BF16, tag="cn")
            nc.vector.tensor_scalar_mul(cn[:], ccb[:], rc[:])

            # --- transpose cc (bf16) on PE, pack [ccT | sq], AllGather ---
            t12 = pp.tile([TP, 2 * TP], BF16, tag="t12")
            nc.tensor.transpose(t12[:, 0:TP], ccb[:, 0:TP], EYE[:])
            nc.tensor.transpose(t12[:, TP : 2 * TP], ccb[:, TP : 2 * TP], EYE[:])
            ct_ext = wp.tile([TP, AG_COLS], FP8, tag="ct_ext")
            nc.scalar.copy(ct_ext[:, 0 : 2 * TP], t12[:])
            nc.vector.tensor_copy(ct_ext[:, 2 * TP : AG_COLS], sqc[:])

            ag_in = dp.tile([TP, AG_COLS], FP8)
            nc.sync.dma_start(ag_in[:], ct_ext[:])
            ag_out = dp.tile([TP * NCORES, AG_COLS], FP8, addr_space="Shared")
            if stage != "noag":
                nc.gpsimd.collective_compute(
                    "AllGather",
                    AluOp.bypass,
                    replica_groups=[list(range(NCORES))],
                    ins=[ag_in.opt()],
                    outs=[ag_out.opt()],
                )

            # --- G matmul operands ---
            l0 = wp.tile([TP, TP], FP8, tag="l0")
            nc.scalar.mul(l0[:], t12[:, 0:TP], -2.0)
            l1 = wp.tile([TP, TP], FP8, tag="l1")
            nc.scalar.mul(l1[:], t12[:, TP : 2 * TP], -2.0)

            ago = ag_out[:].rearrange("(r p) m -> p r m", p=TP)
            NQ = 2
            RQ = NCORES // NQ
            rqs = []
            for q in range(NQ):
                rq = wp.tile([TP, RQ, 2 * TP], FP8, tag=f"rhs{q}")
                eng = nc.sync if q % 2 == 0 else nc.scalar
                eng.dma_start(rq[:], ago[:, q * RQ : (q + 1) * RQ, 0 : 2 * TP])
                rqs.append(rq)
            rhs2 = wp.tile([1, C], FP8, tag="rhs2")
            nc.scalar.dma_start(rhs2[:], ag_out[:, 2 * TP : AG_COLS])

            # --- per-partition partial sums land in fin ---
            fin = wp.tile([TP, 3], F32, tag="fin")

            # --- dist_pc: dots[p,t] = x_p,t . cn_class(p,t). The matching cn
            # row for each sample (partition p <- cn[16t + p//8]) comes from a
            # broadcast-selector PE matmul over the full cn (base partition 0):
            # wbig[:, NT+t, :] has B[k, s] = (k == 16t + s//8). ---
            dots = wp.tile([TP, NT], F32, tag="dots")
            for t in range(NT) if stage != "nodpc" else []:
                ep = pp.tile([TP, D], F32, tag="ep", bufs=2)
                nc.tensor.matmul(
                    ep[:], WBIG[:, NT + t, :], cn[:],
                    start=True, stop=True,
                )
                scr3 = wp.tile([TP, D], F32, tag="scr3", bufs=NT)
                nc.vector.tensor_tensor(scr3[:], Xt(t), ep[:], AluOp.mult)
                nc.vector.tensor_reduce(
                    dots[:, t : t + 1], scr3[:],
                    axis=mybir.AxisListType.X, op=AluOp.add,
                )
            if stage == "nodpc":
                nc.vector.memset(dots[:], 0.5)
            d1 = wp.tile([TP, NT], F32, tag="d1")
            nc.vector.tensor_tensor(d1[:], dots[:], rinv[:], AluOp.mult)
            d2 = wp.tile([TP, NT], F32, tag="d2")
            nc.vector.tensor_scalar(
                d2[:], d1[:], -2.0, 2.0, AluOp.mult, AluOp.add
            )
            d3 = wp.tile([TP, NT], F32, tag="d3")
            nc.vector.tensor_scalar_max(d3[:], d2[:], 0.0)
            dpc = wp.tile([TP, NT], F32, tag="dpc")
            nc.scalar.activation(dpc[:], d3[:], Act.Sqrt, accum_out=fin[:, 0:1])

            # --- hinge rows: G = -2 ccT_local.T @ ccT_full + sq_col; the row
            # term sq_r (+eps) rides in as the activation bias. ---
            sqce = wp.tile([TP, 1], F32, tag="sqce")
            nc.vector.tensor_scalar_add(sqce[:], sqc[:], 1e-12)
            QW = C // NQ
            qsums = wp.tile([TP, NQ], F32, tag="qsums")
            for q in range(NQ) if stage != "nog" else []:
                cs = slice(q * QW, (q + 1) * QW)
                gp = pp.tile([TP, QW], F32, tag="gp", bufs=2)
                nc.tensor.matmul(
                    gp[:], l0[:], rqs[q][:, :, 0:TP], start=True, stop=False
                )
                nc.tensor.matmul(
                    gp[:], l1[:], rqs[q][:, :, TP : 2 * TP],
                    start=False, stop=False,
                )
                nc.tensor.matmul(
                    gp[:], ones_b[:], rhs2[:, cs], start=False, stop=True
                )
                th = wp.tile([TP, QW], F32, tag="th")
                nc.vector.tensor_scalar(
                    th[:], gp[:], sqce[:], 0.0, AluOp.add, AluOp.max
                )
                sh = wp.tile([TP, QW], BF16, tag="sh")
                nc.scalar.activation(sh[:], th[:], Act.Sqrt)
                vh = wp.tile([TP, QW], BF16, tag="vh")
                nc.scalar.activation(
                    vh[:], sh[:], Act.Relu,
                    bias=b_m2[:], scale=-1.0,
                    accum_out=qsums[:, q : q + 1],
                )

            if stage == "nog":
                nc.vector.memset(qsums[:], 0.0)
            nc.vector.tensor_reduce(
                fin[:, 1:2], qsums[:],
                axis=mybir.AxisListType.X, op=AluOp.add,
            )
            nc.vector.memset(fin[:, 2:3], 0.0)
            # per-partition partials go straight out; host sums the 128 rows
            nc.sync.dma_start(out[:], fin[:])

    nc.finalize()
    return nc


def _aux_inputs() -> dict:
    s = np.arange(TP)
    wbig = np.zeros((TP, 2 * NT, TP), ml_dtypes.bfloat16)
    for t in range(NT):
        wbig[s, t, t * CPT + s // K] = 1.0 / K
        wbig[t * CPT + s // K, NT + t, s] = 1.0
    eyeb = np.eye(TP, dtype=ml_dtypes.bfloat16)
    return {"wbig": wbig, "eyeb": eyeb}


def _run_device(inputs: np.ndarray, trace: bool = False, **kw):
    if "nc" not in _CACHE:
        _CACHE["nc"] = _build_nc()
    nc = _CACHE["nc"]
    aux = _aux_inputs()
    in_maps = [
        {"x": np.ascontiguousarray(inputs[r * NL : (r + 1) * NL]), **aux}
        for r in range(NCORES)
    ]
    return run_bass_kernel_spmd(nc, in_maps, list(range(NCORES)), trace=trace, **kw)


def _finish(results) -> tuple:
    parts = np.stack(
        [np.asarray(r["partials"], np.float64).sum(axis=0) for r in results]
    )
    dpc_sum = parts[:, 0].sum()
    an_sum = parts[:, 1].sum() + parts[:, 2].sum() - C * MARGIN2
    dist_pc_mean = dpc_sum / N
    dist_an_mean = an_sum * K / (N - K) / C
    loss = dist_pc_mean + dist_an_mean
    return (
        np.float32(loss),
        np.float32(dist_pc_mean),
        np.float32(dist_an_mean),
    )


def _numpy_fallback(inputs: np.ndarray, targets: np.ndarray) -> tuple:
    # Generic-targets path mirroring reference.py exactly (float64 numpy).
    x = inputs.astype(np.float64)
    n = x.shape[0]
    num_classes = n // K
    x = x / np.linalg.norm(x, axis=1, keepdims=True)
    sums = np.zeros((num_classes, x.shape[1]))
    np.add.at(sums, targets, x)
    counts = np.zeros((num_classes, 1))
    np.add.at(counts, targets, 1.0)
    class_centers = sums / np.maximum(counts, 1)
    centers = class_centers[targets]
    centers_n = centers / np.linalg.norm(centers, axis=1, keepdims=True)
    dist_pc = np.sqrt(np.sum((x - centers_n) ** 2, axis=1))
    dist_pc = np.maximum(dist_pc - 0.0, 0.0)
    sq = np.sum(centers**2, axis=1)
    anchors = np.arange(0, n, K)
    g = centers[anchors] @ centers.T
    dist = np.sqrt(np.maximum(sq[anchors][:, None] + sq[None, :] - 2.0 * g, 1e-12))
    neg = (targets[anchors][:, None] != targets[None, :]).astype(np.float64)
    vals = np.maximum(MARGIN2 - dist, 0.0) * neg
    dist_an = vals.sum(axis=1) / neg.sum(axis=1)
    dpc_m, dan_m = dist_pc.mean(), dist_an.mean()
    return (np.float32(dpc_m + dan_m), np.float32(dpc_m), np.float32(dan_m))


def kernel(inputs: np.ndarray, targets: np.ndarray) -> tuple:
    inputs = np.ascontiguousarray(np.asarray(inputs, np.float32))
    targets = np.asarray(targets)
    if not np.array_equal(
        targets.astype(np.int64), np.arange(N, dtype=np.int64) // K
    ):
        return _numpy_fallback(inputs, targets)
    results = _run_device(inputs).results
    return _finish(results)
